# revision 35
# baseline (speedup 1.0000x reference)
# Multi-head attention (B=2, T=2048, D=1024, H=16) on 8 TRN2 NeuronCores.
#
# Sharding: tensor-parallel over heads. Each core owns 2 heads (a 128-wide
# slice of the hidden dim): it computes its q/k/v projection slice, full
# attention for its 4 (batch, head) pairs, and a partial output projection
# over its slice of the contraction. The 8 partial outputs are summed on the
# host (the TP all-reduce, done as part of unsharding), plus the output bias.
#
# All matmul operands are bf16 (PSUM accumulation stays fp32): rel tolerance
# is 2e-2 and bf16 keeps us ~2.5e-3, while halving DMA/SBUF traffic and
# letting weight loads overlap matmul streaming.
#
# Layouts (per core):
#   xT   [D=1024, B*T=4096]   x transposed so the contraction dim is on SBUF
#                             partitions for the projection matmuls.
#   qT/kT [128, 4096]         head-dim on partitions (2 heads stacked), token
#                             on free axis -> S^T tiles come out of the PE
#                             directly with softmax's reduction on the free
#                             axis of P^T's consumer.
#   v    [128tok, 32kt, 2h, 65]  natural [token, dim] layout per k-tile with a
#                             ones column appended: the ctx matmul then yields
#                             the softmax denominator for free in row 64.
#   ctxT [128, 4096]          both heads' normalized context stacked on
#                             partitions (h0 rows 0-63, h1 rows 64-127) so the
#                             output projection contracts K=128 in one matmul
#                             per tile.
import sys
import types

import numpy as np


def _install_ntff_hook_shim():
    """This image's `antenv` lacks `axon_hooks`, which bass_utils imports
    unconditionally when tracing is requested (e.g. BASS_TRACE=1). Provide
    the module and register the ctypes NTFF hook the way trn_boot would."""
    try:
        import antenv.axon_hooks  # noqa: F401

        return
    except ImportError:
        pass
    try:
        import antenv
    except ImportError:
        return
    mod = types.ModuleType("antenv.axon_hooks")
    _h = [None]
    mod.set_axon_ntff_profile_hook = lambda h: _h.__setitem__(0, h)
    mod.get_axon_ntff_profile_hook = lambda: _h[0]
    sys.modules["antenv.axon_hooks"] = mod
    antenv.axon_hooks = mod
    try:
        from trn_agent_boot.trn_boot import _ntff_profile_via_ctypes

        mod.set_axon_ntff_profile_hook(
            _ntff_profile_via_ctypes("/opt/axon/libaxon_pjrt.so")
        )
    except Exception:
        pass


_install_ntff_hook_shim()

import ml_dtypes

import concourse.bass as bass
import concourse.mybir as mybir
from concourse.bass_utils import run_bass_kernel_spmd
from concourse.tile import TileContext

B, T, D, H = 2, 2048, 1024, 16
HD = D // H          # 64
NCORES = 8
BT = B * T           # 4096
E = D // NCORES      # 128 = per-core slice of hidden dim (2 heads)
HPC = E // HD        # 2 heads per core

F32 = mybir.dt.float32
BF = mybir.dt.bfloat16
AF = mybir.ActivationFunctionType

TCH = 512            # token chunk for projections / q chunks
NTCH = BT // TCH     # 8
NKT = BT // 128      # 32 token tiles of 128
KTB = T // 128       # 16 k-tiles per batch


def build_nc():
    nc = bass.Bass()

    xT = nc.dram_tensor("xT", [D, BT], BF, kind="ExternalInput")
    wqT = nc.dram_tensor("wqT", [D, E], BF, kind="ExternalInput")
    wkT = nc.dram_tensor("wkT", [D, E], BF, kind="ExternalInput")
    wvT = nc.dram_tensor("wvT", [D, E], BF, kind="ExternalInput")
    bq = nc.dram_tensor("bq", [E, 1], F32, kind="ExternalInput")
    bk = nc.dram_tensor("bk", [E, 1], F32, kind="ExternalInput")
    bv = nc.dram_tensor("bv", [E, 1], F32, kind="ExternalInput")
    wo2 = nc.dram_tensor("wo2", [E, D], BF, kind="ExternalInput")
    ones64 = nc.dram_tensor("ones64", [128, HD], BF, kind="ExternalInput")
    ident128 = nc.dram_tensor("ident128", [128, 128], BF, kind="ExternalInput")
    out = nc.dram_tensor("out", [BT, D], BF, kind="ExternalOutput")

    with TileContext(nc) as tc:
        with (
            nc.allow_low_precision(reason="bf16 matmuls are deliberate"),
            tc.tile_pool(name="const", bufs=1) as cpool,
            tc.tile_pool(name="pers", bufs=1) as pers,
            tc.tile_pool(name="work", bufs=2) as work,
            tc.tile_pool(name="psum", bufs=2, space="PSUM") as psum,
        ):
            # ---- constants -------------------------------------------------
            # DMA order matters at startup: wq + x chunks 0-1 first so the
            # first projection matmuls start ASAP; wo_sb (first needed by the
            # out-projection much later) goes last.
            wq_sb = cpool.tile([128, D // 128, E], BF, name="wq_sb")
            wk_sb = cpool.tile([128, D // 128, E], BF, name="wk_sb")
            wv_sb = cpool.tile([128, D // 128, E], BF, name="wv_sb")
            xts = [
                work.tile(
                    [128, D // 128, 2 * TCH], BF, name="xt", tag="xt", bufs=2
                )
                for _ in range(NTCH // 2)
            ]

            def dma_xt(i):
                nc.sync.dma_start(
                    xts[i],
                    xT[:, bass.ts(i, 2 * TCH)].rearrange(
                        "(n p) m -> p n m", p=128
                    ),
                )

            nc.sync.dma_start(wq_sb, wqT.rearrange("(n p) m -> p n m", p=128))
            dma_xt(0)
            bq_sb = cpool.tile([E, 1], F32, name="bq_sb")
            bk_sb = cpool.tile([E, 1], F32, name="bk_sb")
            bv_sb = cpool.tile([E, 1], F32, name="bv_sb")
            nc.sync.dma_start(bq_sb, bq[:, :])
            nc.sync.dma_start(bk_sb, bk[:, :])
            nc.sync.dma_start(bv_sb, bv[:, :])
            nc.sync.dma_start(wk_sb, wkT.rearrange("(n p) m -> p n m", p=128))
            nc.sync.dma_start(wv_sb, wvT.rearrange("(n p) m -> p n m", p=128))
            dma_xt(1)
            ident = cpool.tile([128, 128], BF, name="ident")
            nc.sync.dma_start(ident, ident128[:, :])
            ones_sb = cpool.tile([128, HD], BF, name="ones_sb")
            nc.sync.dma_start(ones_sb, ones64[:, :])
            wo_sb = cpool.tile([E, D], BF, name="wo_sb")
            nc.sync.dma_start(wo_sb, wo2[:, :])

            # ---- persistent activations -----------------------------------
            qT = pers.tile([E, BT], BF, name="qT")
            kT = pers.tile([E, BT], BF, name="kT")
            v = pers.tile([128, NKT, HPC, HD + 1], BF, name="v")
            ctxT = pers.tile([128, BT], BF, name="ctxT")
            nc.sync.dma_start(v[:, :, :, HD], ones64[:, : NKT * HPC])

            # ---- phase A: QKV projections ---------------------------------
            for t in range(NTCH):
                cols = bass.ts(t, TCH)
                xt = xts[t // 2]
                xoff = (t % 2) * TCH
                if t in (2, 4):
                    dma_xt(t // 2 + 1)
                for w_sb, b_sb, dst in (
                    (wq_sb, bq_sb, qT),
                    (wk_sb, bk_sb, kT),
                    (wv_sb, bv_sb, None),
                ):
                    ps = psum.tile([128, TCH], F32, name="ps_mm", tag="mm", bufs=2)
                    for d in range(D // 128):
                        nc.tensor.matmul(
                            ps,
                            lhsT=w_sb[:, d, :],
                            rhs=xt[:, d, xoff : xoff + TCH],
                            start=(d == 0),
                            stop=(d == D // 128 - 1),
                        )
                    if dst is not None:
                        nc.scalar.activation(
                            dst[:, cols], ps, AF.Identity, bias=b_sb, scale=1.0
                        )
                    else:
                        vt = work.tile([128, TCH], BF, name="vt", tag="vt", bufs=2)
                        nc.scalar.activation(vt, ps, AF.Identity, bias=b_sb, scale=1.0)
                        # transpose v back to [token, dim] layout, 128 at a time
                        for i in range(TCH // 128):
                            kt_idx = t * (TCH // 128) + i
                            tp = psum.tile(
                                [128, 128], BF, name="tp", tag="s", bufs=2
                            )
                            nc.tensor.transpose(tp, vt[:, bass.ts(i, 128)], ident)
                            for h in range(HPC):
                                nc.vector.tensor_copy(
                                    v[:, kt_idx, h, 0:HD], tp[:, bass.ts(h, HD)]
                                )

            # ---- phases B+C: attention + output projection, pipelined ------
            # Per k-tile, both heads' S^T matmuls are row-tiled (T0/T8) so
            # they run concurrently on the PE; both land in one [128, 1024]
            # PSUM tile. Exp runs on the scalar engine except every 4th
            # k-tile, which uses a Schraudolph bit-trick exp on the DVE
            # (bits16(e^x) ~= round(x*128/ln2 + 16233) viewed as bf16).
            # ctx matmuls are pipelined one 2-k-tile block behind.
            #
            # Softmax normalization is deferred: per (h, qc) the ctx psum is
            # staged to SBUF (cs, bf16) and its denominator row is DMA'd into
            # one den_all row; per *batch* a single reciprocal serves all 8
            # rows, and the scale-multiplies + output projection of batch b
            # run interleaved with batch b+1's attention so the PE never
            # idles long enough to re-throttle.
            # 4 of 16 k-tiles' exps run on the DVE (Schraudolph), spread so
            # each slots between scalar-engine exps.
            EXP_DVE_KTS = {3, 7, 11, 15}
            EXP_A = 128.0 / float(np.log(2.0)) / 8.0   # folds the 1/8 scale
            EXP_B = 16249.0  # calibrated for zero mean bias vs exact exp
            BLK = 2
            NBLK = KTB // BLK
            NQC = T // TCH          # q chunks per batch
            NST = B * NQC           # global q-chunk steps
            den_alls = {}
            css = {}

            def attn_qchunk(b, qc, weave=()):
                # `weave`: list of closures (trailing normalize / projection
                # work) emitted one per k-tile block so their PE/DVE/ACT ops
                # interleave with this chunk's attention instead of bunching
                # up at a phase boundary.
                weave = list(weave)
                q0 = b * T + qc * TCH
                cps = []
                for h in range(HPC):
                    cp = psum.tile(
                        [HD + 1, TCH], F32, name=f"cp{h}", tag=f"ctx{h}", bufs=1
                    )
                    cps.append(cp)
                pts = {}
                for blk in range(NBLK + 1):
                    if weave:
                        weave.pop(0)()
                    if blk < NBLK:
                        for kt in range(blk * BLK, (blk + 1) * BLK):
                            k0 = b * T + kt * 128
                            sp = psum.tile(
                                [128, HPC * TCH], F32, name="sp", tag="s", bufs=2
                            )
                            for h in range(HPC):
                                he = bass.ts(h, HD)
                                nc.tensor.matmul(
                                    sp[:, bass.ts(h, TCH)],
                                    lhsT=kT[he, k0 : k0 + 128],
                                    rhs=qT[he, q0 : q0 + TCH],
                                    start=True,
                                    stop=True,
                                )
                            pt = work.tile(
                                [128, HPC * TCH], BF, name="pt", tag="pt",
                                bufs=8,
                            )
                            if kt in EXP_DVE_KTS:
                                nc.vector.tensor_scalar(
                                    pt.bitcast(mybir.dt.int16), sp,
                                    EXP_A, EXP_B,
                                    op0=mybir.AluOpType.mult,
                                    op1=mybir.AluOpType.add,
                                )
                            else:
                                nc.scalar.activation(
                                    pt, sp, AF.Exp, scale=1.0 / 8.0
                                )
                            pts[kt] = pt
                    if blk > 0:
                        for kt in range((blk - 1) * BLK, blk * BLK):
                            for h in range(HPC):
                                nc.tensor.matmul(
                                    cps[h],
                                    lhsT=v[:, b * KTB + kt, h, :],
                                    rhs=pts[kt][:, bass.ts(h, TCH)],
                                    start=(kt == 0),
                                    stop=(kt == KTB - 1),
                                    skip_group_check=True,
                                )
                for op in weave:
                    op()
                # stage ctx+den to SBUF (frees psum); den rows collect into
                # the group window tile at 32-aligned partitions (so the
                # broadcast matmuls can read the recip'd rows directly) via
                # partition-shifting DMA. Copies split across ACT/DVE.
                s = b * NQC + qc
                for h in range(HPC):
                    cs = work.tile(
                        [HD + 1, TCH], BF, name="cs", tag="cs", bufs=8
                    )
                    if h == 0:
                        nc.scalar.activation(cs, cps[h], AF.Copy)
                    else:
                        nc.vector.tensor_copy(cs, cps[h])
                    r = den_row(s, h)
                    nc.sync.dma_start(
                        den_alls[GROUP[s]][r : r + 1, :],
                        cs[HD : HD + 1, :],
                    )
                    css[(s, h)] = cs

            def norm_recip(g):
                # 1/den = exp(-ln(den)) on the scalar engine — Ln and Exp
                # share one ACT table set, and this keeps the 3.3us DVE
                # reciprocal out of the DVE FIFO entirely. The intermediate
                # ln value stays fp32: bf16's ~0.03 step at ln(den)~8 would
                # cost 1.6% in the reciprocal.
                lnt = work.tile([33, TCH], F32, name="lnt", tag="lnt", bufs=2)
                nc.scalar.activation(lnt, den_alls[g], AF.Ln)
                nc.scalar.activation(
                    den_alls[g], lnt, AF.Exp, scale=-1.0
                )

            def norm_mult(s, h):
                q0 = s * TCH
                r = den_row(s, h)
                cs = css.pop((s, h))
                rb = psum.tile([HD, TCH], F32, name="rb", tag="mm", bufs=2)
                nc.tensor.matmul(
                    rb,
                    lhsT=ones_sb[r : r + 1, :],
                    rhs=den_alls[GROUP[s]][r : r + 1, :],
                    start=True,
                    stop=True,
                )
                if h == 0:
                    nc.vector.tensor_tensor(
                        ctxT[0:HD, q0 : q0 + TCH],
                        cs[0:HD, :],
                        rb,
                        op=mybir.AluOpType.mult,
                    )
                else:
                    # h1 lives on partitions 64-127 of ctxT; engines can't
                    # shift partitions, so stage and DMA.
                    ctxs = work.tile(
                        [HD, TCH], BF, name="ctxs", tag="ctxs", bufs=2
                    )
                    nc.vector.tensor_tensor(
                        ctxs,
                        cs[0:HD, :],
                        rb,
                        op=mybir.AluOpType.mult,
                    )
                    nc.sync.dma_start(ctxT[HD:128, q0 : q0 + TCH], ctxs)

            def out_proj_tile(tt):
                # ctxT stacks both heads on partitions -> one K=128 matmul
                # per (token tile, out chunk). The two chunks' po copies go
                # to ACT and DVE, then one 256KB DMA writes the full row.
                trows = bass.ts(tt, 128)
                ob = work.tile([128, D], BF, name="ob", tag="ob", bufs=3)
                for nch in range(D // TCH):
                    po = psum.tile([128, TCH], F32, name="po", tag="mm", bufs=2)
                    nc.tensor.matmul(
                        po,
                        lhsT=ctxT[:, trows],
                        rhs=wo_sb[:, bass.ts(nch, TCH)],
                        start=True,
                        stop=True,
                    )
                    nc.vector.tensor_copy(ob[:, bass.ts(nch, TCH)], po)
                nc.sync.dma_start(out[trows, :], ob)

            # each step has its own den window (rows at partitions {0, 32} —
            # matmul operands may only start at 0/32/64, and quadrant 3 is
            # off limits). recip(s) weaves into attn(s+1); norm_mult(s) into
            # attn(s+2), a full step after its recip, so the PE's in-order
            # queue never stalls on a freshly-queued reciprocal.
            GROUP = list(range(NST))

            def den_row(s, h):
                return 32 * h

            for g in range(NST):
                den_alls[g] = work.tile(
                    [33, TCH], BF, name=f"den_all{g}", tag=f"den{g}", bufs=1
                )
                nc.vector.memset(den_alls[g], 1.0)

            # sliding pipeline over 8 global q-chunk steps: attention leads,
            # normalize + output projection trail by 2 steps, woven one op
            # per k-tile block of the leading attention chunk.
            TPS = TCH // 128        # token tiles per step

            nop = lambda: None

            def trailing_ops(s):
                # lag-1: step s-1's recip leads (slot 0), its norm_mults sit
                # at slots 5-6 (past the recip's ACT latency so the PE's
                # in-order queue doesn't stall on the broadcast matmul), and
                # its projection tiles close out the step.
                ops = []
                if s >= 1:
                    p = s - 1
                    ops.append(lambda g=p: norm_recip(g))
                    ops.extend([nop] * 4)
                    for h in range(HPC):
                        ops.append(lambda p=p, h=h: norm_mult(p, h))
                    for tt in range(p * TPS, (p + 1) * TPS):
                        ops.append(lambda tt=tt: out_proj_tile(tt))
                return ops

            for s in range(NST):
                attn_qchunk(s // NQC, s % NQC, weave=trailing_ops(s))
            s = NST - 1
            norm_recip(s)
            for h in range(HPC):
                norm_mult(s, h)
            for tt in range(s * TPS, (s + 1) * TPS):
                out_proj_tile(tt)

    _split_matmul_waits(nc)
    return nc


def _split_matmul_waits(nc):
    """This walrus allows only one sync wait per engine instruction (and none
    on fp32/f32r InstMatmult, whose embedded S3_LW carries the wait slot).
    Move excess waits onto InstEventSemaphore instructions (capacity 2)
    inserted just before the owner in the same engine stream — sequencer
    dispatch is in-order, so semantics are unchanged."""
    ctr = 0
    for f in nc.m.functions:
        for blk in f.blocks:
            out = []
            for inst in blk.instructions:
                si = inst.sync_info
                if (
                    si is not None
                    and not isinstance(inst, mybir.InstEventSemaphore)
                    and len(si.on_wait) > 1
                ):
                    waits = list(si.on_wait)
                    keep = [waits.pop(0)]
                    for i in range(0, len(waits), 2):
                        ev = mybir.InstEventSemaphore(name=f"I-exwait-{ctr}")
                        ctr += 1
                        ev.engine = inst.engine
                        ev.sync_info = mybir.SyncInfo(
                            on_wait=waits[i : i + 2], on_update=[]
                        )
                        nc.register_instruction(ev)
                        out.append(ev)
                    si.on_wait = keep
                out.append(inst)
            blk.instructions[:] = out


_CACHE = {}


def _get_nc():
    if "nc" not in _CACHE:
        _CACHE["nc"] = build_nc()
    return _CACHE["nc"]


def make_in_maps(x, w_qkv, b_qkv, w_out):
    bf = ml_dtypes.bfloat16
    x = np.ascontiguousarray(np.asarray(x, np.float32)).reshape(BT, D)
    w_qkv = np.asarray(w_qkv, np.float32)
    b_qkv = np.asarray(b_qkv, np.float32)
    w_out = np.asarray(w_out, np.float32)

    xT = np.ascontiguousarray(x.T).astype(bf)  # [D, BT]
    wq, wk, wv = w_qkv[0:D], w_qkv[D : 2 * D], w_qkv[2 * D : 3 * D]
    bqs, bks, bvs = b_qkv[0:D], b_qkv[D : 2 * D], b_qkv[2 * D : 3 * D]
    ones = np.ones((128, HD), bf)
    ident = np.eye(128, dtype=np.float32).astype(bf)

    in_maps = []
    for c in range(NCORES):
        rs = slice(E * c, E * (c + 1))
        # wo2[r, o] = w_out[o, E*c + r]  (r = h*64+hd packs both heads on K)
        wo_c = np.ascontiguousarray(w_out[:, rs].T).astype(bf)
        in_maps.append(
            {
                "xT": xT,
                "wqT": np.ascontiguousarray(wq[rs].T).astype(bf),
                "wkT": np.ascontiguousarray(wk[rs].T).astype(bf),
                "wvT": np.ascontiguousarray(wv[rs].T).astype(bf),
                "bq": np.ascontiguousarray(bqs[rs])[:, None],
                "bk": np.ascontiguousarray(bks[rs])[:, None],
                "bv": np.ascontiguousarray(bvs[rs])[:, None],
                "wo2": wo_c,
                "ones64": ones,
                "ident128": ident,
            }
        )
    return in_maps


def _combine(results, b_out):
    acc = results[0]["out"].astype(np.float32)
    for r in results[1:]:
        acc += r["out"].astype(np.float32)
    acc += np.asarray(b_out, np.float32)[None, :]
    return acc.reshape(B, T, D)


def kernel(x, w_qkv, b_qkv, w_out, b_out):
    in_maps = make_in_maps(x, w_qkv, b_qkv, w_out)
    res = run_bass_kernel_spmd(_get_nc(), in_maps, core_ids=list(range(NCORES)))
    return _combine(res.results, b_out)


def kernel_traced(x, w_qkv, b_qkv, w_out, b_out):
    """Like kernel() but profiles the run; returns (output, exec_time_ns)."""
    in_maps = make_in_maps(x, w_qkv, b_qkv, w_out)
    res = run_bass_kernel_spmd(
        _get_nc(), in_maps, core_ids=list(range(NCORES)), trace=True
    )
    return _combine(res.results, b_out), res.exec_time_ns
